# revision 28
# baseline (speedup 1.0000x reference)
"""GatedGraphNN Trainium2 kernel: 8-core SPMD, node-sharded, chunked AllGather.

v2 design (baseline 1.94ms, from 5.17ms v1):
  - messages = h[col] @ W_msg.T + b_msg ; agg = segsum(messages, row). Linearity:
    agg @ W_ih.T = raw @ (W_ih W_msg).T + outer(deg, W_ih b_msg), raw = segsum(h[col]).
    So no per-edge matmul: gather h[col] (bf16), segment-sum via one-hot matmuls on PE,
    then fused dense GRU with W_c = W_ih @ W_msg.
  - Cells are (tile T, src-residue r): slots sorted by (dest-quarter, col), padded to
    128 multiples (pad gathers row 0). 4 SWDGE queues (queue_num=res); ALL SWDGE ops
    must be issued in strict queue round-robin (DMASW sem lanes lock per queue).
  - Step 0 gathers read from a replicated full-x input (no initial AllGather).

v3 changes (1.87-1.94ms; steady step ~375us = desc-gen ~225us + AG overflow):
  - h_full/bounce double-buffered by step parity: the AllGather of step s writes
    h_fulls[(s+1)%2], removing the WAR hazard against step-s gather reads, so AG
    chunks can launch mid-compute.
  - AllGather split into 4 unequal chunks (AG_TILES) launched as their bounce
    tiles complete; the h_full row layout (_rowof) is chunk-major to match.
  - single_packet=False on gathers: 896-desc single packets hogged SDMA engines
    at packet-granularity arbitration and starved the collective's queues.
  - one batched one-hot is_equal per tile (mm ids contiguous across its 4 cells);
    the op cost is dominated by per-op overhead, not columns.
  Known limits (measured): fused dma_gather crashes HW above ~1024 idx regardless
  of dynamic_dma_scratch_size; AG bus_bw drops 2-3x when overlapped with gather
  DMA (HBM/engine contention), so AG wall ~250us/step is the binding resource
  alongside Pool-serial desc-gen (~2.5ns/idx effective across 4 queues).
"""

import numpy as np
import ml_dtypes

BF16 = ml_dtypes.bfloat16
N, H, STEPS, NCORES = 100000, 128, 5, 8
NPAD = 102400
SHARD = NPAD // NCORES          # 12800
NT = SHARD // 512               # 25 dense tiles of 512 dests
NK = 5                          # gather source streams = AllGather chunks
NCELL = NT * NK                 # (T, chunk) cells
# AllGather chunks of 5 tiles each; the gather stream of residue k reads ONLY
# chunk k's tensor, so step s+1 gathers for stream k depend only on chunk k's
# AllGather. Chunk row count 5*512*8=20480 <= int16 range for gather indices.
AG_TILES = [0, 5, 10, 15, 20, 25]
AG_ROWS = [t * 512 for t in AG_TILES]
NAG = len(AG_TILES) - 1
# reversed tile order: the chunk AllGathered FIRST in step s (its tiles
# processed first) is chunk NAG-1, which step s+1's banded k-descending gather
# emission needs FIRST; chunk 0 (AllGathered last) is needed last.
TILE_ORDER = [t for k in range(NAG - 1, -1, -1)
              for t in range(AG_TILES[k], AG_TILES[k + 1])]


def _rowof(col):
    """h_full linear row for global node id (chunk-major AG layout)."""
    col = np.asarray(col)
    c = col // SHARD
    i = col % SHARD
    k = np.searchsorted(AG_ROWS, i, side="right") - 1
    b0 = np.take(AG_ROWS, k)
    b1 = np.take(AG_ROWS, np.minimum(k + 1, NAG))
    return NCORES * b0 + c * (b1 - b0) + (i - b0)


def _preprocess(edge_index):
    """Host-side tables for the (T, res)-cell layout."""
    row = np.asarray(edge_index[0]).astype(np.int64)
    col = np.asarray(edge_index[1]).astype(np.int64)
    core = row // SHARD
    rloc = row - core * SHARD
    T = rloc // 512
    offt = rloc % 512                      # dest offset within tile 0..511
    q = offt // 128                        # dest quarter 0..3
    off7 = offt % 128
    hrow = _rowof(col)
    iloc = col % SHARD
    res = np.searchsorted(AG_ROWS, iloc, side="right") - 1
    gidx = hrow - NCORES * np.take(AG_ROWS, res)   # row within chunk tensor
    cell = T * NK + res

    cnt = np.zeros((NCORES, NCELL), np.int64)
    np.add.at(cnt, (core, cell), 1)
    nchunks = np.maximum(1, (cnt.max(axis=0) + 127) // 128)   # [NCELL]
    cellchunk0 = np.zeros(NCELL + 1, np.int64)
    cellchunk0[1:] = np.cumsum(nchunks)
    TOTC = int(cellchunk0[-1])             # total static chunks

    deg = np.zeros((NCORES, SHARD), np.float32)
    np.add.at(deg, (core, rloc), 1.0)

    # slot assignment: per (core, cell) sorted by (q, col); slots are
    # cell-local positions, trailing pad = -1.
    order = np.lexsort((gidx, q, cell, core))
    core_s, cell_s, q_s, off7_s, gidx_s = (
        core[order], cell[order], q[order], off7[order], gidx[order])

    # pad slots gather row 0 (idx 0): trailing -1 trimming is unsafe because the
    # NX ring accounting uses the untrimmed register count while the Q7 writes
    # the trimmed count, desynchronizing the descriptor ring.
    idxflat = np.zeros((NCORES, TOTC * 128), np.int16)
    # per-edge static slot + chunk
    key = core_s * NCELL + cell_s
    bounds = np.flatnonzero(np.diff(key)) + 1
    starts = np.concatenate([[0], bounds])
    ends = np.concatenate([bounds, [len(key)]])
    pos = np.empty(len(key), np.int64)
    for st, en in zip(starts, ends):
        pos[st:en] = np.arange(en - st)
    slot_global = cellchunk0[cell_s] * 128 + pos       # static slot id
    chunk_global = slot_global // 128
    part = slot_global % 128
    idxflat[core_s, slot_global] = gidx_s.astype(np.int16)

    # mm list: per cell, chunk-local c ascending, q ascending; present if any
    # core has an edge with that (chunk, q).
    pres = np.zeros((TOTC, 4), bool)
    pres[chunk_global, q_s] = True
    mm_of = np.full((TOTC, 4), -1, np.int64)
    mm_entries = []            # (cell, chunk_local, q)
    for cell_i in range(NCELL):
        c0 = int(cellchunk0[cell_i])
        for c in range(int(nchunks[cell_i])):
            for qq in range(4):
                if pres[c0 + c, qq]:
                    mm_of[c0 + c, qq] = len(mm_entries)
                    mm_entries.append((cell_i, c, qq))
    NMM = len(mm_entries)

    offs_mm = np.full((NCORES, 128, NMM), 999.0, np.float32)
    mm_idx_per_edge = mm_of[chunk_global, q_s]
    offs_mm[core_s, part, mm_idx_per_edge] = off7_s

    # per-tile build structure: list of (r, chunk_local, q, mm_idx) in emission
    # order + start/stop flags per (T, q).
    tiles = []
    for Ti in range(NT):
        entries = []
        # quarter-major emission: each PSUM column region's accumulation group
        # is a contiguous run of matmuls (interleaved start/stop groups
        # produced wrong results on hardware).
        for qq in range(4):
            for r in range(NK):
                cell_i = Ti * NK + r
                c0 = int(cellchunk0[cell_i])
                for c in range(int(nchunks[cell_i])):
                    mi = mm_of[c0 + c, qq]
                    if mi >= 0:
                        entries.append([r, c, qq, int(mi), False, False])
        for qq in range(4):
            qe = [e for e in entries if e[2] == qq]
            qe[0][4] = True
            qe[-1][5] = True
        tiles.append(entries)

    # wrap idx into 16 partitions, replicate to 128
    idx16 = np.zeros((NCORES, 128, TOTC * 8), np.int16)
    for c in range(NCORES):
        w = idxflat[c].reshape(TOTC * 8, 16).T
        idx16[c] = np.tile(w, (8, 1))

    own_slots = (np.ceil(cnt / 128) * 128).sum() / NCORES
    return dict(idx16=idx16, offs_mm=offs_mm.astype(BF16), deg=deg,
                nchunks=nchunks, cellchunk0=cellchunk0, TOTC=TOTC, NMM=NMM,
                tiles=tiles, static_slots=TOTC * 128, own_slots=own_slots)


def _build(pp):
    import concourse.bass as bass
    import concourse.bacc as bacc
    import concourse.mybir as mybir
    import concourse.tile as tile
    from concourse.bass import broadcast_tensor_aps

    nchunks = pp["nchunks"]
    cellchunk0 = pp["cellchunk0"]
    TOTC, NMM, tiles = pp["TOTC"], pp["NMM"], pp["tiles"]

    dt = mybir.dt
    AF = mybir.ActivationFunctionType
    OP = mybir.AluOpType
    nc = bacc.Bacc(num_devices=NCORES, num_swdge_queues=4,
                   dynamic_dma_scratch_size=32768)
    RG = [list(range(NCORES))]

    x_T = nc.dram_tensor("x_T", [H, SHARD], dt.bfloat16, kind="ExternalInput")
    xch_d = [nc.dram_tensor(f"xch{k}",
                            [NCORES * (AG_ROWS[k + 1] - AG_ROWS[k]), H],
                            dt.bfloat16, kind="ExternalInput")
             for k in range(NAG)]
    idx_d = nc.dram_tensor("idx", [128, TOTC * 8], dt.int16, kind="ExternalInput")
    offs_d = nc.dram_tensor("offs", [128, NMM], dt.bfloat16, kind="ExternalInput")
    deg_d = nc.dram_tensor("deg", [1, SHARD], dt.bfloat16, kind="ExternalInput")
    wct_d = nc.dram_tensor("wct", [H, 3 * H], dt.bfloat16, kind="ExternalInput")
    whht_d = nc.dram_tensor("whht", [H, 3 * H], dt.bfloat16, kind="ExternalInput")
    v3_d = nc.dram_tensor("v3", [1, 3 * H], dt.bfloat16, kind="ExternalInput")
    bias_d = nc.dram_tensor("bias", [H, 4], dt.float32, kind="ExternalInput")
    iota_d = nc.dram_tensor("iota", [H, H], dt.bfloat16, kind="ExternalInput")
    idn_d = nc.dram_tensor("idn", [H, H], dt.bfloat16, kind="ExternalInput")
    idnf_d = nc.dram_tensor("idnf", [H, H], dt.float32, kind="ExternalInput")
    out_d = nc.dram_tensor("out", [SHARD, H], dt.float32, kind="ExternalOutput")

    # double-buffered: step s gathers read h_fulls[s % 2]; the AllGather of
    # step s writes h_fulls[(s + 1) % 2], so it has no WAR hazard against the
    # in-flight gather reads and can overlap the compute phase.
    h_chs = [[nc.dram_tensor(f"hch{p}_{k}",
                             [NCORES * (AG_ROWS[k + 1] - AG_ROWS[k]), H],
                             dt.bfloat16, kind="Internal", addr_space="Shared")
              for k in range(NAG)] for p in range(2)]
    bounces = [nc.dram_tensor(f"bounce{i}", [SHARD, H], dt.bfloat16,
                              kind="Internal") for i in range(2)]

    with tile.TileContext(nc) as tc:
        with (
            tc.tile_pool(name="res", bufs=1) as res,
            tc.tile_pool(name="gath", bufs=4) as gpool,
            tc.tile_pool(name="oh", bufs=2) as ohpool,
            tc.tile_pool(name="agg", bufs=2) as apool,
            tc.tile_pool(name="epi", bufs=2) as epool,
            tc.tile_pool(name="stg", bufs=2) as spool,
            tc.tile_pool(name="pseg", bufs=2, space="PSUM") as pseg,
            tc.tile_pool(name="pden", bufs=1, space="PSUM") as pden,
            tc.tile_pool(name="ptr", bufs=1, space="PSUM") as ptr,
        ):
            def ld(dram, shape, dtype, name):
                t = res.tile(shape, dtype, tag=name)
                nc.sync.dma_start(t[:], dram[:, :])
                return t

            idx_sb = ld(idx_d, [128, TOTC * 8], dt.int16, "idx")
            offs_sb = ld(offs_d, [128, NMM], dt.bfloat16, "offs")
            deg_sb = ld(deg_d, [1, SHARD], dt.bfloat16, "deg")
            wct = ld(wct_d, [H, 3 * H], dt.bfloat16, "wct")
            whht = ld(whht_d, [H, 3 * H], dt.bfloat16, "whht")
            v3 = ld(v3_d, [1, 3 * H], dt.bfloat16, "v3")
            bias = ld(bias_d, [H, 4], dt.float32, "bias")
            iota = ld(iota_d, [H, H], dt.bfloat16, "iota")
            idn = ld(idn_d, [H, H], dt.bfloat16, "idn")
            idnf = ld(idnf_d, [H, H], dt.float32, "idnf")

            hT = [res.tile([H, SHARD], dt.bfloat16, tag=f"hT{i}", name=f"hT{i}")
                  for i in range(2)]
            nc.sync.dma_start(hT[0][:], x_T[:, :])

            b_r = bias[:, 0:1]
            b_z = bias[:, 1:2]
            b_in = bias[:, 2:3]
            b_hn = bias[:, 3:4]

            maxch = int(nchunks.max())
            maxnmm = max(len(t) for t in tiles)
            # pre-touch gather buffers so trimmed (-1) pad slots never hold
            # inf/nan bit patterns (stale SBUF x 0 one-hot is still computed).
            warm = []
            for r in range(4):
                for _ in range(4):
                    g = gpool.tile([128, maxch, H], dt.bfloat16, tag=f"g{r}")
                    nc.vector.memset(g[:], 0.0)
                    warm.append(g)
            del warm

            for s in range(STEPS):
                hcur, hnxt = hT[s % 2], hT[(s + 1) % 2]
                last = s == STEPS - 1
                srcs = xch_d if s == 0 else h_chs[s % 2]
                bounce = bounces[s % 2]
                done_in_chunk = [0] * NAG
                gpos = 0
                bands = [TILE_ORDER[i:i + 3] for i in range(0, NT, 3)]
                for band in bands:
                  gt = {}
                  # k-descending gather emission within the band: chunk NAG-1
                  # (AllGathered earliest in the previous step) first, chunk 0
                  # (AllGathered last) needed only by the band's final gathers.
                  # queue/tag follow emission order for strict round-robin.
                  for k in range(NAG - 1, -1, -1):
                    for T in band:
                        cell_i = T * NK + k
                        c0 = int(cellchunk0[cell_i])
                        nch = int(nchunks[cell_i])
                        g = gpool.tile([128, maxch, H], dt.bfloat16,
                                       tag=f"g{gpos % 4}")
                        n_idx = nch * 128
                        nc.gpsimd.dma_gather(
                            g[:, 0:nch, :],
                            srcs[k][:, :],
                            idx_sb[:, c0 * 8:(c0 + nch) * 8],
                            n_idx, n_idx, H, queue_num=gpos % 4,
                            single_packet=False)
                        gt[(T, k)] = g
                        gpos += 1
                  for T in band:
                    # one batched one-hot per tile (mm ids contiguous).
                    nmm_T = len(tiles[T])
                    m0_T = min(e[3] for e in tiles[T])
                    oh = ohpool.tile([128, maxnmm * 128], dt.bfloat16,
                                     tag="oh")
                    a_in, b_in2 = broadcast_tensor_aps(
                        offs_sb[:, m0_T:m0_T + nmm_T, None], iota[:, None, :])
                    nc.vector.tensor_tensor(
                        oh[:, 0:nmm_T * 128].rearrange(
                            "p (m f) -> p m f", m=nmm_T),
                        a_in, b_in2, OP.is_equal)

                    ps = pseg.tile([H, 512], dt.float32, tag="pseg")
                    for r, c, qq, mi, st, sp in tiles[T]:
                        nc.tensor.matmul(
                            ps[:, qq * 128:(qq + 1) * 128],
                            gt[(T, r)][:, c, :],
                            oh[:, (mi - m0_T) * 128:(mi - m0_T + 1) * 128],
                            start=st, stop=sp, skip_group_check=True)

                    ragg = apool.tile([H, 512], dt.bfloat16, tag="ragg")
                    nc.scalar.copy(ragg[:], ps[:])

                    hsl = hcur[:, T * 512:(T + 1) * 512]
                    dsl = deg_sb[0:1, T * 512:(T + 1) * 512]
                    p_r = pden.tile([H, 512], dt.float32, tag="p_r")
                    p_z = pden.tile([H, 512], dt.float32, tag="p_z")
                    p_in = pden.tile([H, 512], dt.float32, tag="p_in")
                    p_hn = pden.tile([H, 512], dt.float32, tag="p_hn")
                    nc.tensor.matmul(p_r[:], wct[:, 0:128], ragg[:], start=True, stop=False)
                    nc.tensor.matmul(p_r[:], whht[:, 0:128], hsl, start=False, stop=False)
                    nc.tensor.matmul(p_r[:], v3[0:1, 0:128], dsl, start=False, stop=True)
                    nc.tensor.matmul(p_z[:], wct[:, 128:256], ragg[:], start=True, stop=False)
                    nc.tensor.matmul(p_z[:], whht[:, 128:256], hsl, start=False, stop=False)
                    nc.tensor.matmul(p_z[:], v3[0:1, 128:256], dsl, start=False, stop=True)
                    nc.tensor.matmul(p_in[:], wct[:, 256:384], ragg[:], start=True, stop=False)
                    nc.tensor.matmul(p_in[:], v3[0:1, 256:384], dsl, start=False, stop=True)
                    nc.tensor.matmul(p_hn[:], whht[:, 256:384], hsl, start=True, stop=True)

                    if not last:
                        r_t = epool.tile([H, 512], dt.bfloat16, tag="r")
                        z_t = epool.tile([H, 512], dt.bfloat16, tag="z")
                        ghn = epool.tile([H, 512], dt.bfloat16, tag="ghn")
                        pin = epool.tile([H, 512], dt.bfloat16, tag="pin")
                        t2 = epool.tile([H, 512], dt.bfloat16, tag="t2")
                        pre_n = epool.tile([H, 512], dt.bfloat16, tag="pre_n")
                        nn = epool.tile([H, 512], dt.bfloat16, tag="nn")
                        am = epool.tile([H, 512], dt.bfloat16, tag="am")
                        bm = epool.tile([H, 512], dt.bfloat16, tag="bm")

                        nc.scalar.activation(r_t[:], p_r[:], AF.Sigmoid, bias=b_r)
                        nc.scalar.activation(z_t[:], p_z[:], AF.Sigmoid, bias=b_z)
                        nc.scalar.activation(ghn[:], p_hn[:], AF.Identity, bias=b_hn)
                        nc.scalar.activation(pin[:], p_in[:], AF.Identity, bias=b_in)
                        nc.vector.tensor_tensor(t2[:], r_t[:], ghn[:], OP.mult)
                        nc.vector.tensor_tensor(pre_n[:], t2[:], pin[:], OP.add)
                        nc.scalar.activation(nn[:], pre_n[:], AF.Tanh)
                        nc.vector.tensor_tensor(am[:], hsl, nn[:], OP.subtract)
                        nc.vector.tensor_tensor(bm[:], z_t[:], am[:], OP.mult)
                        hn_sl = hnxt[:, T * 512:(T + 1) * 512]
                        nc.vector.tensor_tensor(hn_sl, bm[:], nn[:], OP.add)
                        stg = spool.tile([128, 4, H], dt.bfloat16, tag="stg")
                        for j in range(4):
                            pt = ptr.tile([128, 128], dt.bfloat16, tag="pt")
                            nc.tensor.transpose(
                                pt[:], hnxt[:, T * 512 + j * 128: T * 512 + (j + 1) * 128],
                                idn[:])
                            nc.scalar.copy(stg[:, j, :], pt[:])
                        nc.sync.dma_start(
                            bounce.rearrange("(t g p) f -> t p g f", p=128, g=4)[T],
                            stg[:])
                        kc = T // 5
                        done_in_chunk[kc] += 1
                        if done_in_chunk[kc] == AG_TILES[kc + 1] - AG_TILES[kc]:
                            r0, r1 = AG_ROWS[kc], AG_ROWS[kc + 1]
                            nc.gpsimd.collective_compute(
                                "AllGather", OP.bypass, replica_groups=RG,
                                ins=[bounce[r0:r1, :]],
                                outs=[h_chs[(s + 1) % 2][kc][:, :]])
                    else:
                        r = epool.tile([H, 512], dt.float32, tag="rf", bufs=1)
                        z = epool.tile([H, 512], dt.float32, tag="zf", bufs=1)
                        ghn = epool.tile([H, 512], dt.float32, tag="ghnf", bufs=1)
                        t2 = epool.tile([H, 512], dt.float32, tag="t2f", bufs=1)
                        pre_n = epool.tile([H, 512], dt.float32, tag="pre_nf", bufs=1)
                        nn = epool.tile([H, 512], dt.float32, tag="nnf", bufs=1)
                        am = epool.tile([H, 512], dt.float32, tag="amf", bufs=1)
                        bm = epool.tile([H, 512], dt.float32, tag="bmf", bufs=1)

                        nc.scalar.activation(r[:], p_r[:], AF.Sigmoid, bias=b_r)
                        nc.scalar.activation(z[:], p_z[:], AF.Sigmoid, bias=b_z)
                        nc.scalar.activation(ghn[:], p_hn[:], AF.Identity, bias=b_hn)
                        nc.vector.scalar_tensor_tensor(
                            t2[:], r[:], 0.0, ghn[:], OP.add, OP.mult)
                        nc.vector.tensor_tensor(pre_n[:], t2[:], p_in[:], OP.add)
                        nc.scalar.activation(nn[:], pre_n[:], AF.Tanh, bias=b_in)
                        nc.vector.tensor_tensor(am[:], hsl, nn[:], OP.subtract)
                        nc.vector.scalar_tensor_tensor(
                            bm[:], z[:], 0.0, am[:], OP.add, OP.mult)
                        hf = epool.tile([H, 512], dt.float32, tag="hf", bufs=2)
                        nc.vector.tensor_tensor(hf[:], bm[:], nn[:], OP.add)
                        stgf = spool.tile([128, 4, H], dt.float32, tag="stgf")
                        for j in range(4):
                            ptf = ptr.tile([128, 128], dt.float32, tag="ptf")
                            nc.tensor.matmul(ptf[:], hf[:, j * 128:(j + 1) * 128],
                                             idnf[:], is_transpose=True)
                            nc.scalar.copy(stgf[:, j, :], ptf[:])
                        nc.sync.dma_start(
                            out_d.rearrange("(t g p) f -> t p g f", p=128, g=4)[T],
                            stgf[:])

    nc.finalize()
    return nc


_CACHE = {}
_last_in_maps = None


def kernel(**inputs):
    x = np.asarray(inputs["x"], np.float32)
    edge_index = np.asarray(inputs["edge_index"])
    W_msg = np.asarray(inputs["W_msg"], np.float32)
    b_msg = np.asarray(inputs["b_msg"], np.float32)
    W_ih = np.asarray(inputs["W_ih"], np.float32)
    W_hh = np.asarray(inputs["W_hh"], np.float32)
    b_ih = np.asarray(inputs["b_ih"], np.float32)
    b_hh = np.asarray(inputs["b_hh"], np.float32)

    pp = _preprocess(edge_index)
    key = (pp["TOTC"], pp["NMM"], tuple(pp["nchunks"].tolist()))
    if key not in _CACHE:
        _CACHE[key] = _build(pp)
    nc = _CACHE[key]

    xp = np.zeros((NPAD, H), np.float32)
    xp[:N] = x
    perm = _rowof(np.arange(NPAD))
    xfull = np.empty((NPAD, H), np.float32)
    xfull[perm] = xp
    xfull = xfull.astype(BF16)
    W_c = W_ih @ W_msg
    v3 = (W_ih @ b_msg).reshape(1, 3 * H)
    bias = np.stack([
        b_ih[0:128] + b_hh[0:128],
        b_ih[128:256] + b_hh[128:256],
        b_ih[256:384],
        b_hh[256:384],
    ], axis=1).astype(np.float32)
    iota = np.broadcast_to(np.arange(H, dtype=np.float32), (H, H)).astype(BF16)
    idn = np.eye(H, dtype=np.float32)

    in_maps = []
    for c in range(NCORES):
        sh = xp[c * SHARD:(c + 1) * SHARD]
        in_maps.append({
            "x_T": np.ascontiguousarray(sh.T).astype(BF16),
            **{f"xch{k}": xfull[NCORES * AG_ROWS[k]:NCORES * AG_ROWS[k + 1]]
               for k in range(NAG)},
            "idx": pp["idx16"][c],
            "offs": pp["offs_mm"][c],
            "deg": pp["deg"][c].reshape(1, SHARD).astype(BF16),
            "wct": np.ascontiguousarray(W_c.T).astype(BF16),
            "whht": np.ascontiguousarray(W_hh.T).astype(BF16),
            "v3": v3.astype(BF16),
            "bias": bias,
            "iota": np.ascontiguousarray(iota),
            "idn": idn.astype(BF16),
            "idnf": idn,
        })

    global _last_in_maps
    _last_in_maps = in_maps
    from concourse.bass_utils import run_bass_kernel_spmd
    res = run_bass_kernel_spmd(nc, in_maps, core_ids=list(range(NCORES)))
    outs = res.results
    full = np.concatenate([outs[c]["out"] for c in range(NCORES)], axis=0)
    return full[:N].astype(np.float32)



# revision 29
# speedup vs baseline: 1.4186x; 1.4186x over previous
"""GatedGraphNN Trainium2 kernel: 8-core SPMD, node-sharded, chunked AllGather.

v2 design (baseline 1.94ms, from 5.17ms v1):
  - messages = h[col] @ W_msg.T + b_msg ; agg = segsum(messages, row). Linearity:
    agg @ W_ih.T = raw @ (W_ih W_msg).T + outer(deg, W_ih b_msg), raw = segsum(h[col]).
    So no per-edge matmul: gather h[col] (bf16), segment-sum via one-hot matmuls on PE,
    then fused dense GRU with W_c = W_ih @ W_msg.
  - Cells are (tile T, src-residue r): slots sorted by (dest-quarter, col), padded to
    128 multiples (pad gathers row 0). 4 SWDGE queues (queue_num=res); ALL SWDGE ops
    must be issued in strict queue round-robin (DMASW sem lanes lock per queue).
  - Step 0 gathers read from a replicated full-x input (no initial AllGather).

v3 changes (1.87-1.94ms; steady step ~375us = desc-gen ~225us + AG overflow):
  - h_full/bounce double-buffered by step parity: the AllGather of step s writes
    h_fulls[(s+1)%2], removing the WAR hazard against step-s gather reads, so AG
    chunks can launch mid-compute.
  - AllGather split into 4 unequal chunks (AG_TILES) launched as their bounce
    tiles complete; the h_full row layout (_rowof) is chunk-major to match.
  - single_packet=False on gathers: 896-desc single packets hogged SDMA engines
    at packet-granularity arbitration and starved the collective's queues.
  - one batched one-hot is_equal per tile (mm ids contiguous across its 4 cells);
    the op cost is dominated by per-op overhead, not columns.
  Known limits (measured): fused dma_gather crashes HW above ~1024 idx regardless
  of dynamic_dma_scratch_size; AG bus_bw drops 2-3x when overlapped with gather
  DMA (HBM/engine contention), so AG wall ~250us/step is the binding resource
  alongside Pool-serial desc-gen (~2.5ns/idx effective across 4 queues).
"""

import numpy as np
import ml_dtypes

BF16 = ml_dtypes.bfloat16
N, H, STEPS, NCORES = 100000, 128, 5, 8
NPAD = 102400
SHARD = NPAD // NCORES          # 12800
NT = SHARD // 512               # 25 dense tiles of 512 dests
NCELL = NT * 4                  # (T, res) cells
# AllGather chunk boundaries, in dense tiles. Geometric split: each chunk
# launches as soon as its tiles' bounce rows exist, so the big early chunks
# overlap the remaining compute and only the small last chunk is serial.
AG_TILES = [0, 10, 16, 21, 25]
AG_ROWS = [t * 512 for t in AG_TILES]
NAG = len(AG_TILES) - 1


def _rowof(col):
    """h_full linear row for global node id (chunk-major AG layout)."""
    col = np.asarray(col)
    c = col // SHARD
    i = col % SHARD
    k = np.searchsorted(AG_ROWS, i, side="right") - 1
    b0 = np.take(AG_ROWS, k)
    b1 = np.take(AG_ROWS, np.minimum(k + 1, NAG))
    return NCORES * b0 + c * (b1 - b0) + (i - b0)


def _preprocess(edge_index):
    """Host-side tables for the (T, res)-cell layout."""
    row = np.asarray(edge_index[0]).astype(np.int64)
    col = np.asarray(edge_index[1]).astype(np.int64)
    core = row // SHARD
    rloc = row - core * SHARD
    T = rloc // 512
    offt = rloc % 512                      # dest offset within tile 0..511
    q = offt // 128                        # dest quarter 0..3
    off7 = offt % 128
    hrow = _rowof(col)
    res = hrow % 4
    gidx = hrow // 4
    cell = T * 4 + res                     # 0..99

    cnt = np.zeros((NCORES, NCELL), np.int64)
    np.add.at(cnt, (core, cell), 1)
    nchunks = np.maximum(1, (cnt.max(axis=0) + 127) // 128)   # [NCELL]
    cellchunk0 = np.zeros(NCELL + 1, np.int64)
    cellchunk0[1:] = np.cumsum(nchunks)
    TOTC = int(cellchunk0[-1])             # total static chunks

    deg = np.zeros((NCORES, SHARD), np.float32)
    np.add.at(deg, (core, rloc), 1.0)

    # slot assignment: per (core, cell) sorted by (q, col); slots are
    # cell-local positions, trailing pad = -1.
    order = np.lexsort((gidx, q, cell, core))
    core_s, cell_s, q_s, off7_s, gidx_s = (
        core[order], cell[order], q[order], off7[order], gidx[order])

    # pad slots gather row 0 (idx 0): trailing -1 trimming is unsafe because the
    # NX ring accounting uses the untrimmed register count while the Q7 writes
    # the trimmed count, desynchronizing the descriptor ring.
    idxflat = np.zeros((NCORES, TOTC * 128), np.int16)
    # per-edge static slot + chunk
    key = core_s * NCELL + cell_s
    bounds = np.flatnonzero(np.diff(key)) + 1
    starts = np.concatenate([[0], bounds])
    ends = np.concatenate([bounds, [len(key)]])
    pos = np.empty(len(key), np.int64)
    for st, en in zip(starts, ends):
        pos[st:en] = np.arange(en - st)
    slot_global = cellchunk0[cell_s] * 128 + pos       # static slot id
    chunk_global = slot_global // 128
    part = slot_global % 128
    idxflat[core_s, slot_global] = gidx_s.astype(np.int16)

    # mm list: per cell, chunk-local c ascending, q ascending; present if any
    # core has an edge with that (chunk, q).
    pres = np.zeros((TOTC, 4), bool)
    pres[chunk_global, q_s] = True
    mm_of = np.full((TOTC, 4), -1, np.int64)
    mm_entries = []            # (cell, chunk_local, q)
    for cell_i in range(NCELL):
        c0 = int(cellchunk0[cell_i])
        for c in range(int(nchunks[cell_i])):
            for qq in range(4):
                if pres[c0 + c, qq]:
                    mm_of[c0 + c, qq] = len(mm_entries)
                    mm_entries.append((cell_i, c, qq))
    NMM = len(mm_entries)

    offs_mm = np.full((NCORES, 128, NMM), 999.0, np.float32)
    mm_idx_per_edge = mm_of[chunk_global, q_s]
    offs_mm[core_s, part, mm_idx_per_edge] = off7_s

    # per-tile build structure: list of (r, chunk_local, q, mm_idx) in emission
    # order + start/stop flags per (T, q).
    tiles = []
    for Ti in range(NT):
        entries = []
        # quarter-major emission: each PSUM column region's accumulation group
        # is a contiguous run of matmuls (interleaved start/stop groups
        # produced wrong results on hardware).
        for qq in range(4):
            for r in range(4):
                cell_i = Ti * 4 + r
                c0 = int(cellchunk0[cell_i])
                for c in range(int(nchunks[cell_i])):
                    mi = mm_of[c0 + c, qq]
                    if mi >= 0:
                        entries.append([r, c, qq, int(mi), False, False])
        for qq in range(4):
            qe = [e for e in entries if e[2] == qq]
            qe[0][4] = True
            qe[-1][5] = True
        tiles.append(entries)

    # wrap idx into 16 partitions, replicate to 128
    idx16 = np.zeros((NCORES, 128, TOTC * 8), np.int16)
    for c in range(NCORES):
        w = idxflat[c].reshape(TOTC * 8, 16).T
        idx16[c] = np.tile(w, (8, 1))

    own_slots = (np.ceil(cnt / 128) * 128).sum() / NCORES
    return dict(idx16=idx16, offs_mm=offs_mm.astype(BF16), deg=deg,
                nchunks=nchunks, cellchunk0=cellchunk0, TOTC=TOTC, NMM=NMM,
                tiles=tiles, static_slots=TOTC * 128, own_slots=own_slots)


def _build(pp):
    import concourse.bass as bass
    import concourse.bacc as bacc
    import concourse.mybir as mybir
    import concourse.tile as tile
    from concourse.bass import broadcast_tensor_aps

    nchunks = pp["nchunks"]
    cellchunk0 = pp["cellchunk0"]
    TOTC, NMM, tiles = pp["TOTC"], pp["NMM"], pp["tiles"]

    dt = mybir.dt
    AF = mybir.ActivationFunctionType
    OP = mybir.AluOpType
    nc = bacc.Bacc(num_devices=NCORES, num_swdge_queues=4,
                   dynamic_dma_scratch_size=32768)
    RG = [list(range(NCORES))]

    x_T = nc.dram_tensor("x_T", [H, SHARD], dt.bfloat16, kind="ExternalInput")
    xfull_d = nc.dram_tensor("xfull", [NPAD, H], dt.bfloat16, kind="ExternalInput")
    idx_d = nc.dram_tensor("idx", [128, TOTC * 8], dt.int16, kind="ExternalInput")
    offs_d = nc.dram_tensor("offs", [128, NMM], dt.bfloat16, kind="ExternalInput")
    deg_d = nc.dram_tensor("deg", [1, SHARD], dt.bfloat16, kind="ExternalInput")
    wct_d = nc.dram_tensor("wct", [H, 3 * H], dt.bfloat16, kind="ExternalInput")
    whht_d = nc.dram_tensor("whht", [H, 3 * H], dt.bfloat16, kind="ExternalInput")
    v3_d = nc.dram_tensor("v3", [1, 3 * H], dt.bfloat16, kind="ExternalInput")
    bias_d = nc.dram_tensor("bias", [H, 4], dt.float32, kind="ExternalInput")
    iota_d = nc.dram_tensor("iota", [H, H], dt.bfloat16, kind="ExternalInput")
    idn_d = nc.dram_tensor("idn", [H, H], dt.bfloat16, kind="ExternalInput")
    idnf_d = nc.dram_tensor("idnf", [H, H], dt.float32, kind="ExternalInput")
    out_d = nc.dram_tensor("out", [SHARD, H], dt.float32, kind="ExternalOutput")

    # double-buffered: step s gathers read h_fulls[s % 2]; the AllGather of
    # step s writes h_fulls[(s + 1) % 2], so it has no WAR hazard against the
    # in-flight gather reads and can overlap the compute phase.
    h_fulls = [nc.dram_tensor(f"h_full{i}", [NPAD, H], dt.bfloat16,
                              kind="Internal", addr_space="Shared")
               for i in range(2)]
    bounces = [nc.dram_tensor(f"bounce{i}", [SHARD, H], dt.bfloat16,
                              kind="Internal") for i in range(2)]

    with tile.TileContext(nc) as tc:
        with (
            tc.tile_pool(name="res", bufs=1) as res,
            tc.tile_pool(name="gath", bufs=3) as gpool,
            tc.tile_pool(name="oh", bufs=2) as ohpool,
            tc.tile_pool(name="agg", bufs=2) as apool,
            tc.tile_pool(name="epi", bufs=2) as epool,
            tc.tile_pool(name="stg", bufs=2) as spool,
            tc.tile_pool(name="pseg", bufs=2, space="PSUM") as pseg,
            tc.tile_pool(name="pden", bufs=1, space="PSUM") as pden,
            tc.tile_pool(name="ptr", bufs=1, space="PSUM") as ptr,
        ):
            def ld(dram, shape, dtype, name):
                t = res.tile(shape, dtype, tag=name)
                nc.sync.dma_start(t[:], dram[:, :])
                return t

            idx_sb = ld(idx_d, [128, TOTC * 8], dt.int16, "idx")
            offs_sb = ld(offs_d, [128, NMM], dt.bfloat16, "offs")
            deg_sb = ld(deg_d, [1, SHARD], dt.bfloat16, "deg")
            wct = ld(wct_d, [H, 3 * H], dt.bfloat16, "wct")
            whht = ld(whht_d, [H, 3 * H], dt.bfloat16, "whht")
            v3 = ld(v3_d, [1, 3 * H], dt.bfloat16, "v3")
            bias = ld(bias_d, [H, 4], dt.float32, "bias")
            iota = ld(iota_d, [H, H], dt.bfloat16, "iota")
            idn = ld(idn_d, [H, H], dt.bfloat16, "idn")
            idnf = ld(idnf_d, [H, H], dt.float32, "idnf")

            hT = [res.tile([H, SHARD], dt.bfloat16, tag=f"hT{i}", name=f"hT{i}")
                  for i in range(2)]
            nc.sync.dma_start(hT[0][:], x_T[:, :])

            b_r = bias[:, 0:1]
            b_z = bias[:, 1:2]
            b_in = bias[:, 2:3]
            b_hn = bias[:, 3:4]

            maxch = int(nchunks.max())
            maxnmm = max(len(t) for t in tiles)
            # pre-touch gather buffers so trimmed (-1) pad slots never hold
            # inf/nan bit patterns (stale SBUF x 0 one-hot is still computed).
            warm = []
            for r in range(4):
                for _ in range(3):
                    g = gpool.tile([128, maxch, H], dt.bfloat16, tag=f"g{r}")
                    nc.vector.memset(g[:], 0.0)
                    warm.append(g)
            del warm

            for s in range(STEPS):
                hcur, hnxt = hT[s % 2], hT[(s + 1) % 2]
                last = s == STEPS - 1
                src_d = xfull_d if s == 0 else h_fulls[s % 2]
                bounce = bounces[s % 2]
                for T in range(NT):
                    gt = {}
                    # one batched one-hot per tile: a tile's mm ids are
                    # contiguous across its 4 cells, and the is_equal cost is
                    # dominated by per-op overhead, not columns.
                    nmm_T = len(tiles[T])
                    m0_T = min(e[3] for e in tiles[T])
                    oh = ohpool.tile([128, maxnmm * 128], dt.bfloat16,
                                     tag="oh")
                    a_in, b_in2 = broadcast_tensor_aps(
                        offs_sb[:, m0_T:m0_T + nmm_T, None], iota[:, None, :])
                    nc.vector.tensor_tensor(
                        oh[:, 0:nmm_T * 128].rearrange(
                            "p (m f) -> p m f", m=nmm_T),
                        a_in, b_in2, OP.is_equal)
                    for r in range(4):
                        cell_i = T * 4 + r
                        c0 = int(cellchunk0[cell_i])
                        nch = int(nchunks[cell_i])
                        g = gpool.tile([128, maxch, H], dt.bfloat16, tag=f"g{r}")
                        n_idx = nch * 128
                        nc.gpsimd.dma_gather(
                            g[:, 0:nch, :],
                            src_d[r::4, :],
                            idx_sb[:, c0 * 8:(c0 + nch) * 8],
                            n_idx, n_idx, H, elem_step=4 * H, queue_num=r,
                            single_packet=False)
                        gt[r] = g

                    ps = pseg.tile([H, 512], dt.float32, tag="pseg")
                    for r, c, qq, mi, st, sp in tiles[T]:
                        nc.tensor.matmul(
                            ps[:, qq * 128:(qq + 1) * 128],
                            gt[r][:, c, :],
                            oh[:, (mi - m0_T) * 128:(mi - m0_T + 1) * 128],
                            start=st, stop=sp, skip_group_check=True)

                    ragg = apool.tile([H, 512], dt.bfloat16, tag="ragg")
                    nc.scalar.copy(ragg[:], ps[:])

                    hsl = hcur[:, T * 512:(T + 1) * 512]
                    dsl = deg_sb[0:1, T * 512:(T + 1) * 512]
                    p_r = pden.tile([H, 512], dt.float32, tag="p_r")
                    p_z = pden.tile([H, 512], dt.float32, tag="p_z")
                    p_in = pden.tile([H, 512], dt.float32, tag="p_in")
                    p_hn = pden.tile([H, 512], dt.float32, tag="p_hn")
                    nc.tensor.matmul(p_r[:], wct[:, 0:128], ragg[:], start=True, stop=False)
                    nc.tensor.matmul(p_r[:], whht[:, 0:128], hsl, start=False, stop=False)
                    nc.tensor.matmul(p_r[:], v3[0:1, 0:128], dsl, start=False, stop=True)
                    nc.tensor.matmul(p_z[:], wct[:, 128:256], ragg[:], start=True, stop=False)
                    nc.tensor.matmul(p_z[:], whht[:, 128:256], hsl, start=False, stop=False)
                    nc.tensor.matmul(p_z[:], v3[0:1, 128:256], dsl, start=False, stop=True)
                    nc.tensor.matmul(p_in[:], wct[:, 256:384], ragg[:], start=True, stop=False)
                    nc.tensor.matmul(p_in[:], v3[0:1, 256:384], dsl, start=False, stop=True)
                    nc.tensor.matmul(p_hn[:], whht[:, 256:384], hsl, start=True, stop=True)

                    if not last:
                        r_t = epool.tile([H, 512], dt.bfloat16, tag="r")
                        z_t = epool.tile([H, 512], dt.bfloat16, tag="z")
                        ghn = epool.tile([H, 512], dt.bfloat16, tag="ghn")
                        pin = epool.tile([H, 512], dt.bfloat16, tag="pin")
                        t2 = epool.tile([H, 512], dt.bfloat16, tag="t2")
                        pre_n = epool.tile([H, 512], dt.bfloat16, tag="pre_n")
                        nn = epool.tile([H, 512], dt.bfloat16, tag="nn")
                        am = epool.tile([H, 512], dt.bfloat16, tag="am")
                        bm = epool.tile([H, 512], dt.bfloat16, tag="bm")

                        nc.scalar.activation(r_t[:], p_r[:], AF.Sigmoid, bias=b_r)
                        nc.scalar.activation(z_t[:], p_z[:], AF.Sigmoid, bias=b_z)
                        nc.scalar.activation(ghn[:], p_hn[:], AF.Identity, bias=b_hn)
                        nc.scalar.activation(pin[:], p_in[:], AF.Identity, bias=b_in)
                        nc.vector.tensor_tensor(t2[:], r_t[:], ghn[:], OP.mult)
                        nc.vector.tensor_tensor(pre_n[:], t2[:], pin[:], OP.add)
                        nc.scalar.activation(nn[:], pre_n[:], AF.Tanh)
                        nc.vector.tensor_tensor(am[:], hsl, nn[:], OP.subtract)
                        nc.vector.tensor_tensor(bm[:], z_t[:], am[:], OP.mult)
                        hn_sl = hnxt[:, T * 512:(T + 1) * 512]
                        nc.vector.tensor_tensor(hn_sl, bm[:], nn[:], OP.add)
                        stg = spool.tile([128, 4, H], dt.bfloat16, tag="stg")
                        for j in range(4):
                            pt = ptr.tile([128, 128], dt.bfloat16, tag="pt")
                            nc.tensor.transpose(
                                pt[:], hnxt[:, T * 512 + j * 128: T * 512 + (j + 1) * 128],
                                idn[:])
                            nc.scalar.copy(stg[:, j, :], pt[:])
                        nc.sync.dma_start(
                            bounce.rearrange("(t g p) f -> t p g f", p=128, g=4)[T],
                            stg[:])
                        if T + 1 in AG_TILES:
                            k = AG_TILES.index(T + 1) - 1
                            r0, r1 = AG_ROWS[k], AG_ROWS[k + 1]
                            nc.gpsimd.collective_compute(
                                "AllGather", OP.bypass, replica_groups=RG,
                                ins=[bounce[r0:r1, :]],
                                outs=[h_fulls[(s + 1) % 2][NCORES * r0:NCORES * r1, :]])
                    else:
                        r = epool.tile([H, 512], dt.float32, tag="rf")
                        z = epool.tile([H, 512], dt.float32, tag="zf")
                        ghn = epool.tile([H, 512], dt.float32, tag="ghnf")
                        t2 = epool.tile([H, 512], dt.float32, tag="t2f", bufs=1)
                        pre_n = epool.tile([H, 512], dt.float32, tag="pre_nf", bufs=1)
                        nn = epool.tile([H, 512], dt.float32, tag="nnf")
                        am = epool.tile([H, 512], dt.float32, tag="amf", bufs=1)
                        bm = epool.tile([H, 512], dt.float32, tag="bmf", bufs=1)

                        nc.scalar.activation(r[:], p_r[:], AF.Sigmoid, bias=b_r)
                        nc.scalar.activation(z[:], p_z[:], AF.Sigmoid, bias=b_z)
                        nc.scalar.activation(ghn[:], p_hn[:], AF.Identity, bias=b_hn)
                        nc.vector.scalar_tensor_tensor(
                            t2[:], r[:], 0.0, ghn[:], OP.add, OP.mult)
                        nc.vector.tensor_tensor(pre_n[:], t2[:], p_in[:], OP.add)
                        nc.scalar.activation(nn[:], pre_n[:], AF.Tanh, bias=b_in)
                        nc.vector.tensor_tensor(am[:], hsl, nn[:], OP.subtract)
                        nc.vector.scalar_tensor_tensor(
                            bm[:], z[:], 0.0, am[:], OP.add, OP.mult)
                        hf = epool.tile([H, 512], dt.float32, tag="hf", bufs=2)
                        nc.vector.tensor_tensor(hf[:], bm[:], nn[:], OP.add)
                        stgf = spool.tile([128, 4, H], dt.float32, tag="stgf")
                        for j in range(4):
                            ptf = ptr.tile([128, 128], dt.float32, tag="ptf")
                            nc.tensor.matmul(ptf[:], hf[:, j * 128:(j + 1) * 128],
                                             idnf[:], is_transpose=True)
                            nc.scalar.copy(stgf[:, j, :], ptf[:])
                        nc.sync.dma_start(
                            out_d.rearrange("(t g p) f -> t p g f", p=128, g=4)[T],
                            stgf[:])

    nc.finalize()
    return nc


_CACHE = {}
_last_in_maps = None


def kernel(**inputs):
    x = np.asarray(inputs["x"], np.float32)
    edge_index = np.asarray(inputs["edge_index"])
    W_msg = np.asarray(inputs["W_msg"], np.float32)
    b_msg = np.asarray(inputs["b_msg"], np.float32)
    W_ih = np.asarray(inputs["W_ih"], np.float32)
    W_hh = np.asarray(inputs["W_hh"], np.float32)
    b_ih = np.asarray(inputs["b_ih"], np.float32)
    b_hh = np.asarray(inputs["b_hh"], np.float32)

    pp = _preprocess(edge_index)
    key = (pp["TOTC"], pp["NMM"], tuple(pp["nchunks"].tolist()))
    if key not in _CACHE:
        _CACHE[key] = _build(pp)
    nc = _CACHE[key]

    xp = np.zeros((NPAD, H), np.float32)
    xp[:N] = x
    perm = _rowof(np.arange(NPAD))
    xfull = np.empty((NPAD, H), np.float32)
    xfull[perm] = xp
    xfull = xfull.astype(BF16)
    W_c = W_ih @ W_msg
    v3 = (W_ih @ b_msg).reshape(1, 3 * H)
    bias = np.stack([
        b_ih[0:128] + b_hh[0:128],
        b_ih[128:256] + b_hh[128:256],
        b_ih[256:384],
        b_hh[256:384],
    ], axis=1).astype(np.float32)
    iota = np.broadcast_to(np.arange(H, dtype=np.float32), (H, H)).astype(BF16)
    idn = np.eye(H, dtype=np.float32)

    in_maps = []
    for c in range(NCORES):
        sh = xp[c * SHARD:(c + 1) * SHARD]
        in_maps.append({
            "x_T": np.ascontiguousarray(sh.T).astype(BF16),
            "xfull": xfull,
            "idx": pp["idx16"][c],
            "offs": pp["offs_mm"][c],
            "deg": pp["deg"][c].reshape(1, SHARD).astype(BF16),
            "wct": np.ascontiguousarray(W_c.T).astype(BF16),
            "whht": np.ascontiguousarray(W_hh.T).astype(BF16),
            "v3": v3.astype(BF16),
            "bias": bias,
            "iota": np.ascontiguousarray(iota),
            "idn": idn.astype(BF16),
            "idnf": idn,
        })

    global _last_in_maps
    _last_in_maps = in_maps
    from concourse.bass_utils import run_bass_kernel_spmd
    res = run_bass_kernel_spmd(nc, in_maps, core_ids=list(range(NCORES)))
    outs = res.results
    full = np.concatenate([outs[c]["out"] for c in range(NCORES)], axis=0)
    return full[:N].astype(np.float32)

